# revision 6
# baseline (speedup 1.0000x reference)
"""Trainium2 Bass kernel for nn_LSTMModel (B=4096, T=365, F=32, H=50).

Data-parallel over batch: 8 cores x 512 rows, no collectives.

On-chip design (per core, raw Bass engine blocks with explicit semaphores —
each instruction carries at most ONE wait; ordering beyond that is encoded in
semaphore thresholds, which are transitive by construction):

- Transposed state layout: h^T, c^T are [64, 512] bf16 (H padded 50->64),
  batch on the free dim, so the recurrence needs no per-step transposes.
- Host-side prep (free): x transposed/cast to [T, 33, 512] bf16 with a ones
  row so the LSTM bias folds into the x-part matmul; weights padded,
  gate-reordered, cast to bf16.
- Per step, gate preacts land in one PSUM pair [128, 1024] f32 via 4 bf16
  matmuls (h-part start=True, then x-part accumulates): cols 0:512 = {f|i},
  cols 512:1024 = {o|cand} (vertical split at partition 64).
- One sigmoid ACT op covers f, i, o (cand's relu is fused into the
  scalar_tensor_tensor multiply; the sigmoid of the cand half is junk and
  unused).
- DVE: fc = sig(f)*c ; ic = relu(g_cand)*sig(i) at partition base 64
  (PSUM operand exempt from the equal-base rule); GPSIMD realigns ic to
  base 0; c = fc + ic ; h = relu(c)*sig(o).
"""

import numpy as np
import ml_dtypes

import concourse.bass as bass
from concourse import mybir
from concourse.bass_utils import run_bass_kernel_spmd

F32 = mybir.dt.float32
BF16 = mybir.dt.bfloat16
ALU = mybir.AluOpType
ACTF = mybir.ActivationFunctionType

H = 50
HP = 64
F = 32
FP = 33
T = 365
B = 4096
NCORES = 8
BPC = B // NCORES      # 512
XC = 4                 # timesteps per x DMA chunk
RING = 8               # x chunk ring buffers


def build_lstm(nc: bass.Bass, t_steps: int = T):
    nchunks = (t_steps + XC - 1) // XC

    xt = nc.dram_tensor("xt", [t_steps, FP, BPC], BF16, kind="ExternalInput").ap()
    wx = nc.dram_tensor("wx", [FP, 256], BF16, kind="ExternalInput").ap()
    wh = nc.dram_tensor("wh", [HP, 256], BF16, kind="ExternalInput").ap()
    dw = nc.dram_tensor("dw", [HP, 1], BF16, kind="ExternalInput").ap()
    db = nc.dram_tensor("db", [1, 1], F32, kind="ExternalInput").ap()
    out = nc.dram_tensor("out", [BPC], F32, kind="ExternalOutput").ap()

    wx_sb = nc.alloc_sbuf_tensor("wx_sb", [FP, 256], BF16).ap()
    wh_sb = nc.alloc_sbuf_tensor("wh_sb", [HP, 256], BF16).ap()
    dw_sb = nc.alloc_sbuf_tensor("dw_sb", [HP, 1], BF16).ap()
    db_sb = nc.alloc_sbuf_tensor("db_sb", [1, 1], F32).ap()
    x_sb = nc.alloc_sbuf_tensor("x_sb", [FP, RING, XC, BPC], BF16).ap()
    s_sb = nc.alloc_sbuf_tensor("s_sb", [128, 2 * BPC], BF16).ap()
    h_sb = nc.alloc_sbuf_tensor("h_sb", [HP, BPC], BF16).ap()
    c_sb = nc.alloc_sbuf_tensor("c_sb", [HP, BPC], BF16).ap()
    fc_sb = nc.alloc_sbuf_tensor("fc_sb", [HP, BPC], BF16).ap()
    ic_sb = nc.alloc_sbuf_tensor("ic_sb", [128, BPC], BF16).ap()
    icr_sb = nc.alloc_sbuf_tensor("icr_sb", [HP, BPC], BF16).ap()
    o_sb = nc.alloc_sbuf_tensor("o_sb", [1, BPC], F32).ap()

    g_ps = nc.alloc_psum_tensor("g_ps", [128, 1024], F32).ap()
    od_ps = nc.alloc_psum_tensor("od_ps", [1, BPC], F32).ap()

    NW = 4  # weight dmas

    with (
        nc.Block() as block,
        nc.semaphore("dma_sem") as dma_sem,
        nc.semaphore("pe_sem") as pe_sem,
        nc.semaphore("act_sem") as act_sem,
        nc.semaphore("dve_sem") as dve_sem,
        nc.semaphore("gps_sem") as gps_sem,
    ):

        @block.sync
        def _(eng: bass.BassEngine):
            eng.dma_start(out=wx_sb, in_=wx).then_inc(dma_sem, 16)
            eng.dma_start(out=wh_sb, in_=wh).then_inc(dma_sem, 16)
            eng.dma_start(out=dw_sb, in_=dw).then_inc(dma_sem, 16)
            eng.dma_start(out=db_sb, in_=db).then_inc(dma_sem, 16)
            for k in range(nchunks):
                if k >= RING:
                    # don't overwrite a chunk until its steps are consumed
                    eng.wait_ge(pe_sem, (k - RING + 1) * XC)
                # chain on prior DMA completions so dma_sem thresholds
                # identify a prefix (DMA completions are otherwise unordered)
                eng.wait_ge(dma_sem, 16 * (NW + k))
                ns = min(XC, t_steps - k * XC)
                eng.dma_start(
                    out=x_sb[:, k % RING, :ns, :],
                    in_=xt[k * XC:k * XC + ns].transpose([1, 0, 2]),
                ).then_inc(dma_sem, 16)
            # final output store
            eng.wait_ge(dma_sem, 16 * (NW + nchunks))
            eng.wait_ge(act_sem, t_steps + 1)
            eng.dma_start(out=out.unsqueeze(0), in_=o_sb).then_inc(dma_sem, 16)

        @block.tensor
        def _(eng: bass.BassEngine):
            eng.wait_ge(dma_sem, 16 * NW)  # weights resident
            for t in range(t_steps):
                # WAR: dve >= 2 + 4t means h(t-1), ic(t-1), c(t-1) all done
                eng.wait_ge(dve_sem, 2 + 4 * t)
                eng.matmul(g_ps[:, 0:BPC], lhsT=wh_sb[:, 0:128], rhs=h_sb,
                           start=True, stop=False)
                if t % XC == 0:
                    eng.wait_ge(dma_sem, 16 * (NW + t // XC + 1))
                xs = x_sb[:, (t // XC) % RING, t % XC, :]
                eng.matmul(g_ps[:, 0:BPC], lhsT=wx_sb[:, 0:128], rhs=xs,
                           start=False, stop=True)
                eng.matmul(g_ps[:, BPC:2 * BPC], lhsT=wh_sb[:, 128:256], rhs=h_sb,
                           start=True, stop=False)
                eng.matmul(g_ps[:, BPC:2 * BPC], lhsT=wx_sb[:, 128:256], rhs=xs,
                           start=False, stop=True).then_inc(pe_sem)
            # final dense (h @ dense_w); dve wait implies everything upstream
            eng.wait_ge(dve_sem, 2 + 4 * t_steps)
            eng.matmul(od_ps, lhsT=dw_sb, rhs=h_sb, start=True,
                       stop=True).then_inc(pe_sem)

        @block.scalar
        def _(eng: bass.BassEngine):
            for t in range(t_steps):
                eng.wait_ge(pe_sem, t + 1)
                # sig over both banks: sig(f)@p0,c0 sig(i)@p64,c0
                #                      sig(o)@p0,c1 junk@p64,c1
                eng.activation(s_sb, g_ps, ACTF.Sigmoid).then_inc(act_sem)
            eng.wait_ge(pe_sem, t_steps + 1)
            eng.activation(o_sb, od_ps, ACTF.Identity,
                           bias=db_sb).then_inc(act_sem)

        @block.vector
        def _(eng: bass.BassEngine):
            eng.memset(h_sb, 0.0).then_inc(dve_sem)
            eng.memset(c_sb, 0.0).then_inc(dve_sem)
            for t in range(t_steps):
                eng.wait_ge(act_sem, t + 1)
                # fc = sig(f) * c
                eng.tensor_mul(fc_sb, s_sb[0:64, 0:BPC], c_sb).then_inc(dve_sem)
                # ic@64 = relu(g_cand) * sig(i)
                eng.scalar_tensor_tensor(
                    out=ic_sb[64:128], in0=g_ps[64:128, BPC:2 * BPC], scalar=0.0,
                    in1=s_sb[64:128, 0:BPC], op0=ALU.max,
                    op1=ALU.mult).then_inc(dve_sem)
                # c = fc + ic (after gpsimd realign)
                eng.wait_ge(gps_sem, t + 1)
                eng.tensor_add(c_sb, fc_sb, icr_sb).then_inc(dve_sem)
                # h = relu(c) * sig(o)
                eng.scalar_tensor_tensor(
                    out=h_sb, in0=c_sb, scalar=0.0, in1=s_sb[0:64, BPC:2 * BPC],
                    op0=ALU.max, op1=ALU.mult).then_inc(dve_sem)

        @block.gpsimd
        def _(eng: bass.BassEngine):
            for t in range(t_steps):
                eng.wait_ge(dve_sem, 2 + 4 * t + 2)
                eng.tensor_copy(out=icr_sb, in_=ic_sb[64:128]).then_inc(gps_sem)


def _prep_inputs(x, kernel, rec_kernel, bias, dense_w, dense_b):
    """Host-side: shard, transpose, pad, reorder gates, fold bias, cast."""
    bf16 = ml_dtypes.bfloat16

    def reorder(w_np, out_rows):
        rows = w_np.shape[0]
        o = np.zeros((out_rows, 256), np.float32)
        blocks = {"i": w_np[:, 0:H], "f": w_np[:, H:2 * H],
                  "c": w_np[:, 2 * H:3 * H], "o": w_np[:, 3 * H:4 * H]}
        # cols 0:128 -> [f | i]; cols 128:256 -> [o | cand]
        for dst, name in ((0, "f"), (64, "i"), (128, "o"), (192, "c")):
            o[:rows, dst:dst + H] = blocks[name]
        return o

    wx = reorder(kernel, FP)
    wx[F, :] = reorder(bias[None, :], 1)[0]     # ones-row of x carries the bias
    wh = reorder(rec_kernel, HP)
    dwp = np.zeros((HP, 1), np.float32)
    dwp[:H] = dense_w
    dbp = np.asarray(dense_b, np.float32).reshape(1, 1)

    in_maps = []
    for core in range(NCORES):
        xc = x[core * BPC:(core + 1) * BPC]                 # [BPC, T, F]
        xtc = np.ascontiguousarray(xc.transpose(1, 2, 0))   # [T, F, BPC]
        xt = np.empty((T, FP, BPC), np.float32)
        xt[:, :F, :] = xtc
        xt[:, F, :] = 1.0
        in_maps.append(dict(xt=xt.astype(bf16), wx=wx.astype(bf16),
                            wh=wh.astype(bf16), dw=dwp.astype(bf16), db=dbp))
    return in_maps


def kernel(x, kernel, rec_kernel, bias, dense_w, dense_b):
    x = np.asarray(x, np.float32)
    in_maps = _prep_inputs(x, np.asarray(kernel, np.float32),
                           np.asarray(rec_kernel, np.float32),
                           np.asarray(bias, np.float32),
                           np.asarray(dense_w, np.float32),
                           np.asarray(dense_b, np.float32))
    nc = bass.Bass("TRN2", target_bir_lowering=False, debug=False)
    build_lstm(nc)
    res = run_bass_kernel_spmd(nc, in_maps, list(range(NCORES)))
    out = np.concatenate([np.asarray(r["out"], np.float32) for r in res.results])
    return out.reshape(B, 1)


if __name__ == "__main__":
    from concourse.bass_interp import CoreSim

    t_small = 12
    rng = np.random.default_rng(0)
    x = rng.normal(size=(BPC, T, F)).astype(np.float32)
    kern = (rng.normal(size=(F, 4 * H)) * 0.1).astype(np.float32)
    rec = (rng.normal(size=(H, 4 * H)) * 0.1).astype(np.float32)
    bias_np = (rng.normal(size=(4 * H,)) * 0.1).astype(np.float32)
    dwn = (rng.normal(size=(H, 1)) * 0.1).astype(np.float32)
    dbn = np.zeros((1,), np.float32)

    def ref_np(x, kern, rec, bias, dw, db, tt):
        xg = np.einsum("btf,fg->btg", x[:, :tt], kern) + bias
        h = np.zeros((x.shape[0], H), np.float32)
        c = np.zeros((x.shape[0], H), np.float32)
        sig = lambda v: 1.0 / (1.0 + np.exp(-v))
        for t in range(tt):
            g = xg[:, t] + h @ rec
            i = sig(g[:, :H]); f = sig(g[:, H:2 * H])
            cand = np.maximum(g[:, 2 * H:3 * H], 0); o = sig(g[:, 3 * H:])
            c = f * c + i * cand
            h = o * np.maximum(c, 0)
        return h @ dw + db

    expected = ref_np(x, kern, rec, bias_np, dwn, dbn, t_small)

    bf16 = ml_dtypes.bfloat16
    wxp = np.zeros((FP, 256), np.float32)
    xfull = np.concatenate([np.pad(x[:, :t_small], ((0, 0), (0, T - t_small), (0, 0)))] * NCORES)
    m0 = _prep_inputs(xfull, kern, rec, bias_np, dwn, dbn)[0]
    m0["xt"] = m0["xt"][:t_small]

    nc = bass.Bass("TRN2", target_bir_lowering=False, debug=False, detect_race_conditions=False)
    build_lstm(nc, t_steps=t_small)
    sim = CoreSim(nc)
    for k, v in m0.items():
        sim.tensor(k)[:] = v
    sim.simulate()
    got = np.asarray(sim.tensor("out")).reshape(-1, 1)
    err = np.abs(got - expected).max() / (np.abs(expected).max() + 1e-9)
    print(f"CoreSim t={t_small}: rel err {err:.3e}")
